# revision 4
# baseline (speedup 1.0000x reference)
"""Trainium2 Bass kernel: per-sample 64-bin histogram + normalize + tiny MLP.

Input  grad_map [128, 512, 512] f32, W1 [32,64], b1 [32], W2 [128,32], b2 [128]
Output [128, 128] f32 = relu(hist_norm @ W1.T + b1) @ W2.T + b2
Sharding: pure data parallel over batch across 8 cores (16 samples/core).

Strategy (64 bins = 8 hi x 8 lo, joint counts via one TensorE Gram/sample):
- idx = round(x*64/255 - 0.5) i16 (VE pass, 2x DVE mode), lo = idx & 7
  (VE, 4x); 7 hi step planes (idx >= 8a), 7 lo planes (lo >= b), and a
  ones plane (memset once per double buffer), all bf16 at DVE 4x rate
- planes are engine-balanced across VE / ScalarE / GpSimd; the ScalarE
  planes use saturated Sigmoid(64*(idx-8a+0.5)), which is exactly 0/1
  in bf16 for integer idx, so no sign correction is needed; hi planes
  1 and 4 are split by j-range (H1_VE / H4_ACT) for exact balance
- Gram: C[(b,g),(a,g')] += SL[:,j].T @ SH[:,j] over 256 f-interleaved
  chunks of 8 columns -> PSUM [64, 64]; mask out g != g' (VE; GpSimd
  cannot read PSUM), halving-add ladder over g' (idle GpSimd), then
  e8.T @ cred -> C2[b, a] per sample into a shared PSUM tile column
- tail: h1 = sum_a What[:,a,:].T @ C2[:,(s,a)] with BOTH second-
  difference matrices and the 1/N normalization folded into the W1
  weights host-side; relu; W2 matmul; biases; DMA out

Scheduling (emission order = per-engine execution order):
- per iteration: dma(s+2); planes(s); idx(s+1); Gram(s); epilogue(s-1)
  so no engine queue head ever waits on a slower producer
- PE work/sample (~6.9us) slightly exceeds every plane engine, keeping
  the PE continuously busy at full p-state (it is the pipeline pacer)
- all weights ride ONE blob DMA dispatched first; sample 0 is built in
  two halves behind two half-DMAs (shorter fill, its plane-4 share on
  VE); the whole MLP tail runs for samples 0..14 overlapping the last
  Gram, so only sample 15 rides the serial late chain
"""

import numpy as np

import concourse.bacc as bacc
import concourse.mybir as mybir
from concourse.mybir import AluOpType
from concourse.tile import TileContext
from concourse.bass_utils import run_bass_kernel_spmd

HIST_BINS = 64
VMAX = 255.0
SCALE = float(np.float32(HIST_BINS / VMAX))
B, H, W = 128, 512, 512
N_CORES = 8
SPC = B // N_CORES            # 16 samples per core
NPEL = H * W                  # 262144
P = 128
PF = NPEL // P                # 2048 free elems per partition
G = 8                         # f-columns per Gram matmul
NMM = PF // G                 # 128 matmuls per sample

F32 = mybir.dt.float32
I16 = mybir.dt.int16
BF16 = mybir.dt.bfloat16

POOL_HI = (2, 3)              # hi planes always built on GpSimd
ACT_HI = (5, 6, 7)            # hi planes always built on ScalarE
ALT_HI = 4                    # hi plane split ACT/VE by j-range
H1_VE = 240                   # VE builds j 0:H1_VE of hi plane 1, Pool rest
H4_ACT = 155                  # ACT builds j 0:H4_ACT of hi plane 4, VE rest
CUSHION_MM = 0                # extra PE matmuls/sample so the PE stays the
                              # (p-state-hot) pipeline pacer

# weight blob column layout: [mask 128 | e8 8 | abias 8 | dtb 8 | b2 1 | rest]
# rows 0..63 of 'rest' hold W1.T ([8a+b, j] 32 cols), rows 64..95 hold W2.T
# ([j, 128] needs 128 cols) - W2.T gets its own region instead.
BLOB_COLS = 64 + 8 + 8 + 8 + 1 + 256 + 128 + 1  # mask e8 abias dtb b2 w1 w2t b1


def build_kernel():
    nc = bacc.Bacc("TRN2", target_bir_lowering=False)

    x = nc.dram_tensor("x", [SPC, P, PF], F32, kind="ExternalInput")
    wblob = nc.dram_tensor("wblob", [P, BLOB_COLS], F32, kind="ExternalInput")
    y = nc.dram_tensor("y", [P, SPC], F32, kind="ExternalOutput")

    with TileContext(nc) as tc:
        with (
            tc.tile_pool(name="xp", bufs=3) as xp,
            tc.tile_pool(name="idxp", bufs=3) as idxp,
            tc.tile_pool(name="sm", bufs=1) as sm,
            tc.tile_pool(name="wk", bufs=3) as wk,
            tc.tile_pool(name="ps", bufs=3, space="PSUM") as ps,
            tc.tile_pool(name="psc", bufs=1, space="PSUM") as psc,
            tc.tile_pool(name="ps1", bufs=1, space="PSUM") as ps1,
        ):
            # a tiny dummy activation binds the ACT function-table load at
            # t~0 instead of on sample 0's critical path
            dummy_sb = sm.tile([1, 2], F32)
            nc.vector.memset(dummy_sb[:], 0.0)
            nc.scalar.activation(
                dummy_sb[:], dummy_sb[:],
                mybir.ActivationFunctionType.Sigmoid, bias=0.0, scale=1.0,
            )

            idxs = []
            xts = []

            def load_dma(s):
                xt = xp.tile([P, PF], F32, name=f"xt{s}", tag="xt")
                nc.sync.dma_start(out=xt[:], in_=x[s])
                xts.append(xt)

            # weight blob first (ACT planes gate the fill on abias), then
            # sample 0 in two halves, then x(1).
            blob_sb = sm.tile([P, BLOB_COLS], F32)
            nc.sync.dma_start(out=blob_sb[:], in_=wblob[:])
            xt0 = xp.tile([P, PF], F32, name="xt0", tag="xt")
            xts.append(xt0)
            HMM = NMM // 2
            nc.sync.dma_start(out=xt0[:, 0 : PF // 2], in_=x[0][:, 0 : PF // 2])
            nc.sync.dma_start(out=xt0[:, PF // 2 : PF], in_=x[0][:, PF // 2 : PF])
            load_dma(1)
            c0 = 0
            mask_sb = blob_sb[0:64, c0 : c0 + 64]; c0 += 64
            e8_sb = blob_sb[0:64, c0 : c0 + 8]; c0 += 8
            abias_sb = blob_sb[:, c0 : c0 + 8]; c0 += 8
            dtb_sb = blob_sb[0:8, c0 : c0 + 8]; c0 += 8
            b2_sb = blob_sb[:, c0 : c0 + 1]; c0 += 1
            w1_sb = blob_sb[0:8, c0 : c0 + 256].rearrange(
                "b (a j) -> b a j", a=8
            ); c0 += 256   # [b, a, j]
            w2t_sb = blob_sb[0:32, c0 : c0 + 128]; c0 += 128
            b1_sb = blob_sb[0:32, c0 : c0 + 1]; c0 += 1

            # double-buffered step planes; ones plane written once each
            sh_tiles = [
                sm.tile([P, NMM, 8, G], BF16, name=f"sh{i}", tag=f"sh{i}")
                for i in range(2)
            ]
            sl_tiles = [
                sm.tile([P, NMM, 8, G], BF16, name=f"sl{i}", tag=f"sl{i}")
                for i in range(2)
            ]
            for i in range(2):
                nc.gpsimd.memset(sh_tiles[i][:, :, 0, :], 1.0)
                nc.vector.memset(sl_tiles[i][:, :, 0, :], 1.0)

            # C2 for all samples: [b, (s, a)], written by one matmul/sample
            t2b_ps = ps1.tile([8, SPC, 8], F32)

            los = []

            def emit_idx(s, e0=0, e1=PF):
                if len(idxs) <= s:
                    idxs.append(idxp.tile([P, PF], I16, name=f"idx{s}", tag="idx"))
                    los.append(idxp.tile([P, PF], I16, name=f"lo{s}", tag="lo"))
                nc.vector.tensor_scalar(
                    idxs[s][:, e0:e1], xts[s][:, e0:e1], SCALE, 0.5,
                    AluOpType.mult, AluOpType.subtract,
                )
                nc.vector.tensor_scalar(
                    los[s][:, e0:e1], idxs[s][:, e0:e1], 7, None,
                    AluOpType.bitwise_and,
                )

            def emit_planes(s, j0=0, j1=NMM, h4_act=None):
                if h4_act is None:
                    h4_act = H4_ACT
                idx_v = idxs[s][:].rearrange("p (j g) -> p j g", g=G)
                lo_v = los[s][:].rearrange("p (j g) -> p j g", g=G)
                SH = sh_tiles[s % 2]
                SL = sl_tiles[s % 2]
                # hi planes (idx >= 8a) -> SH[:, :, a, :]; planes 1 and 4
                # are split by j-range across engines for exact balance
                s1 = min(max(H1_VE, j0), j1)
                if s1 > j0:
                    nc.vector.tensor_scalar(
                        SH[:, j0:s1, 1, :], idx_v[:, j0:s1, :], 8.0, None,
                        AluOpType.is_ge,
                    )
                if j1 > s1:
                    nc.gpsimd.tensor_scalar(
                        SH[:, s1:j1, 1, :], idx_v[:, s1:j1, :], 8.0, None,
                        AluOpType.is_ge,
                    )
                for a in POOL_HI:
                    nc.gpsimd.tensor_scalar(
                        SH[:, j0:j1, a, :], idx_v[:, j0:j1, :], float(8 * a),
                        None, AluOpType.is_ge,
                    )
                for a in ACT_HI:
                    # saturated sigmoid: exactly 0/1 in bf16 for integer idx
                    nc.scalar.activation(
                        SH[:, j0:j1, a, :],
                        idx_v[:, j0:j1, :],
                        mybir.ActivationFunctionType.Sigmoid,
                        bias=abias_sb[:, a : a + 1],
                        scale=64.0,
                    )
                s4 = min(max(h4_act, j0), j1)
                if s4 > j0:
                    nc.scalar.activation(
                        SH[:, j0:s4, ALT_HI, :],
                        idx_v[:, j0:s4, :],
                        mybir.ActivationFunctionType.Sigmoid,
                        bias=abias_sb[:, ALT_HI : ALT_HI + 1],
                        scale=64.0,
                    )
                if j1 > s4:
                    nc.vector.tensor_scalar(
                        SH[:, s4:j1, ALT_HI, :], idx_v[:, s4:j1, :],
                        float(8 * ALT_HI), None, AluOpType.is_ge,
                    )
                # lo planes (lo >= b), one single-op pass each
                for b in range(1, 8):
                    nc.vector.tensor_scalar(
                        SL[:, j0:j1, b, :], lo_v[:, j0:j1, :], float(b), None,
                        AluOpType.is_ge,
                    )

            cps_tiles = {}

            def emit_gram(s):
                SH = sh_tiles[s % 2]
                SL = sl_tiles[s % 2]
                c_ps = ps.tile([64, 64], F32, tag="cps")
                cps_tiles[s] = c_ps
                for j in range(NMM):
                    nc.tensor.matmul(
                        c_ps[:],
                        SL[:, j].rearrange("p b g -> p (b g)"),
                        SH[:, j].rearrange("p a g -> p (a g)"),
                        start=(j == 0),
                        stop=(j == NMM - 1),
                    )
                if CUSHION_MM:
                    cu_ps = psc.tile([64, 64], F32, tag="cush")
                    for j in range(CUSHION_MM):
                        nc.tensor.matmul(
                            cu_ps[:],
                            SL[:, j].rearrange("p b g -> p (b g)"),
                            SH[:, j].rearrange("p a g -> p (a g)"),
                            start=(j == 0),
                            stop=(j == CUSHION_MM - 1),
                        )

            def emit_epilogue(s, ladder_ve=False):
                c_ps = cps_tiles[s]
                eng = nc.vector if ladder_ve else nc.gpsimd
                # mask g != g' cross terms (VE: GpSimd cannot read PSUM),
                # then halving-add ladder over g' (idle GpSimd; VE for the
                # last sample where the Q7 launches would sit on the tail)
                cm = wk.tile([64, 8, G], F32, tag="cm")
                nc.vector.tensor_tensor(
                    cm[:].rearrange("p a g -> p (a g)"), c_ps[:], mask_sb[:],
                    AluOpType.mult,
                )
                ch4 = wk.tile([64, 8, 4], F32, tag="ch4")
                eng.tensor_tensor(
                    ch4[:], cm[:, :, 0:4], cm[:, :, 4:8], AluOpType.add
                )
                ch2 = wk.tile([64, 8, 2], F32, tag="ch2")
                eng.tensor_tensor(
                    ch2[:], ch4[:, :, 0:2], ch4[:, :, 2:4], AluOpType.add
                )
                cred = wk.tile([64, 8], F32, tag="cred")
                eng.tensor_tensor(
                    cred[:], ch2[:, :, 0], ch2[:, :, 1], AluOpType.add
                )
                # C2[b, a] for this sample straight into the shared PSUM tile
                nc.tensor.matmul(
                    t2b_ps[:, s, :], e8_sb[:], cred[:], start=True, stop=True
                )

            # emission order is engine-queue order: keep VE planes ahead of
            # the next idx, and the PE epilogue of s-1 behind Gram(s), so no
            # engine queue head ever waits on a slower producer.
            # sample 0 in two halves behind two half-DMAs (shorter fill);
            # its share of plane 4 goes to VE (the fill is ACT-gated)
            emit_idx(0, 0, PF // 2)
            emit_planes(0, 0, HMM, h4_act=0)
            emit_idx(0, PF // 2, PF)
            emit_planes(0, HMM, NMM, h4_act=0)
            for s in range(SPC):
                if s + 2 < SPC:
                    load_dma(s + 2)
                if s > 0:
                    emit_planes(s)
                if s + 1 < SPC:
                    emit_idx(s + 1)
                emit_gram(s)
                if s > 0:
                    emit_epilogue(s - 1)

            # ---- tail: the whole MLP runs twice, samples 0..14 overlap
            # the last Gram/epilogue; only sample 15's column rides the
            # serial late chain ----
            t2b_sb = sm.tile([8, SPC, 8], F32)
            h1_ps = ps1.tile([32, SPC], F32)
            h1r_sb = sm.tile([32, SPC], F32)
            out_ps = ps1.tile([P, SPC], F32)
            out_sb = sm.tile([P, SPC], F32)
            SL15 = SPC - 1

            def emit_mlp_tail(s0, s1):
                nc.scalar.activation(
                    t2b_sb[:, s0:s1].rearrange("p s a -> p (s a)"),
                    t2b_ps[:, s0:s1].rearrange("p s a -> p (s a)"),
                    mybir.ActivationFunctionType.Copy,
                    bias=0.0,
                    scale=1.0,
                )
                # h1 = sum_a What[:, a, :].T @ C2[:, (s, a)]; both second-
                # difference matrices and the 1/N are folded into What
                for a in range(8):
                    nc.tensor.matmul(
                        h1_ps[:, s0:s1],
                        w1_sb[:, a, :],
                        t2b_sb[:, s0:s1, a],
                        start=(a == 0),
                        stop=(a == 7),
                    )
                nc.scalar.activation(
                    h1r_sb[:, s0:s1], h1_ps[:, s0:s1],
                    mybir.ActivationFunctionType.Relu, bias=b1_sb, scale=1.0,
                )
                nc.tensor.matmul(
                    out_ps[:, s0:s1], w2t_sb, h1r_sb[:, s0:s1],
                    start=True, stop=True,
                )
                nc.scalar.activation(
                    out_sb[:, s0:s1], out_ps[:, s0:s1],
                    mybir.ActivationFunctionType.Identity, bias=b2_sb, scale=1.0,
                )
                nc.sync.dma_start(out=y[:, s0:s1], in_=out_sb[:, s0:s1])

            emit_mlp_tail(0, SL15)
            emit_epilogue(SL15, ladder_ve=True)
            emit_mlp_tail(SL15, SPC)

    nc.compile()
    return nc


_NC_CACHE = {}


def kernel(grad_map, W1, b1, W2, b2, _trace=False):
    grad_map = np.ascontiguousarray(grad_map, dtype=np.float32)
    W1 = np.asarray(W1, dtype=np.float32)
    b1 = np.asarray(b1, dtype=np.float32)
    W2 = np.asarray(W2, dtype=np.float32)
    b2 = np.asarray(b2, dtype=np.float32)

    if "nc" not in _NC_CACHE:
        _NC_CACHE["nc"] = build_kernel()
    nc = _NC_CACHE["nc"]

    blob = np.zeros((P, BLOB_COLS), np.float32)
    c0 = 0
    # mask[(b,g), (a,g')] = delta_{g,g'}
    blob[0:64, c0 : c0 + 64] = np.kron(
        np.ones((8, 8), np.float32), np.eye(G, dtype=np.float32)
    ); c0 += 64
    # e8[(b,g), b'] = delta_{b,b'}
    blob[0:64, c0 : c0 + 8] = np.kron(
        np.eye(8, dtype=np.float32), np.ones((G, 1), np.float32)
    ); c0 += 8
    # sigmoid bias per hi plane a: 64*(0.5 - 8a)
    blob[:, c0 : c0 + 8] = np.array(
        [64.0 * (0.5 - 8.0 * a) for a in range(8)], np.float32
    )[None, :]; c0 += 8
    # dtb slot kept for layout compatibility (no longer used on-device)
    dbm = np.eye(8, dtype=np.float32) - np.eye(8, k=1, dtype=np.float32)
    blob[0:8, c0 : c0 + 8] = (dbm / np.float32(NPEL)).T; c0 += 8
    blob[:, c0] = b2; c0 += 1
    # What[b, a, j] = sum_{b',a'} DB[b',b] DA[a',a] W1[j, 8a'+b'] / N:
    # h1 = sum_{b,a} What[b,a,j] C2[a,b] equals W1 @ histn (both second
    # differences of C2 folded into the weights)
    w1r = W1.T.reshape(8, 8, 32).transpose(1, 0, 2)      # [b', a', j]
    what = np.einsum("ki,lj,klm->ijm", dbm, dbm, w1r) / np.float32(NPEL)
    blob[0:8, c0 : c0 + 256] = what.reshape(8, 256).astype(np.float32); c0 += 256
    blob[0:32, c0 : c0 + 128] = W2.T; c0 += 128
    blob[0:32, c0] = b1; c0 += 1
    assert c0 == BLOB_COLS

    xs = grad_map.reshape(N_CORES, SPC, P, PF)
    in_maps = [
        {"x": np.ascontiguousarray(xs[c]), "wblob": blob} for c in range(N_CORES)
    ]

    res = run_bass_kernel_spmd(
        nc, in_maps, core_ids=list(range(N_CORES)), trace=_trace
    )
    out = np.concatenate([r["y"].T for r in res.results], axis=0)
    if _trace:
        return out, res
    return out


# revision 5
# speedup vs baseline: 1.0004x; 1.0004x over previous
"""Trainium2 Bass kernel v2: per-sample 64-bin histogram + normalize + tiny MLP.

Input  grad_map [128, 512, 512] f32, W1 [32,64], b1 [32], W2 [128,32], b2 [128]
Output [128, 128] f32 = relu(hist_norm @ W1.T + b1) @ W2.T + b2
Sharding: pure data parallel over batch across 8 cores (16 samples/core).

Strategy (64 bins = 8 hi x 8 lo, joint counts by a TensorE Gram):
- idx = round(x*64/255 - 0.5) i16 (1 VE pass at 2x DVE rate)
- 7 hi step planes (idx >= 8a) and 7 lo planes ((idx&7) >= b, fused
  and+cmp in ONE tensor_scalar) + 1 ones plane (memset once per buffer)
- planes split VE 8.5 / ACT 3.5 / Pool 2 by engine-rate balance; the
  ACT planes use saturated Sigmoid(64*(idx-8a+0.5)) which is exactly
  0/1 in bf16, so no sign-correction anywhere
- Gram: C[(b,g),(a,g')] += SL[:,j].T @ SH[:,j] over 128 f-interleaved
  chunks; mask out g!=g', halving-add ladder over g' (idle Pool), then
  e8.T @ cred -> C2[b, a] per sample written into a shared PSUM tile
- 2nd difference of C2 + MLP tail, all b-major

Scheduling (the critical part, engine queues run in emission order):
- per iteration: dma(s+2); planes(s); idx(s+1); Gram(s); epilogue(s-1)
  so no engine queue head ever waits on a slower producer
- PE work/sample (6.87us) slightly exceeds every plane engine, keeping
  the PE continuously busy at full p-state (it is the pipeline pacer)
- all weights ride ONE blob DMA, dispatched after x(0)/x(1)
"""

import numpy as np

import concourse.bacc as bacc
import concourse.mybir as mybir
from concourse.mybir import AluOpType
from concourse.tile import TileContext
from concourse.bass_utils import run_bass_kernel_spmd

HIST_BINS = 64
VMAX = 255.0
SCALE = float(np.float32(HIST_BINS / VMAX))
B, H, W = 128, 512, 512
N_CORES = 8
SPC = B // N_CORES            # 16 samples per core
NPEL = H * W                  # 262144
P = 128
PF = NPEL // P                # 2048 free elems per partition
G = 8                         # f-columns per Gram matmul
NMM = PF // G                 # 128 matmuls per sample

F32 = mybir.dt.float32
I16 = mybir.dt.int16
BF16 = mybir.dt.bfloat16

POOL_HI = (2, 3)              # hi planes always built on GpSimd
ACT_HI = (5, 6, 7)            # hi planes always built on ScalarE
ALT_HI = 4                    # hi plane split ACT/VE by j-range
H1_VE = 240                   # VE builds j 0:H1_VE of hi plane 1, Pool rest
H4_ACT = 155                  # ACT builds j 0:H4_ACT of hi plane 4, VE rest
CUSHION_MM = 0                # extra PE matmuls/sample so the PE stays the
                              # (p-state-hot) pipeline pacer

# weight blob column layout: [mask 128 | e8 8 | abias 8 | dtb 8 | b2 1 | rest]
# rows 0..63 of 'rest' hold W1.T ([8a+b, j] 32 cols), rows 64..95 hold W2.T
# ([j, 128] needs 128 cols) - W2.T gets its own region instead.
BLOB_COLS = 64 + 8 + 8 + 8 + 1 + 256 + 128 + 1  # mask e8 abias dtb b2 w1 w2t b1


def build_kernel():
    nc = bacc.Bacc("TRN2", target_bir_lowering=False)

    x = nc.dram_tensor("x", [SPC, P, PF], F32, kind="ExternalInput")
    wblob = nc.dram_tensor("wblob", [P, BLOB_COLS], F32, kind="ExternalInput")
    y = nc.dram_tensor("y", [P, SPC], F32, kind="ExternalOutput")

    with TileContext(nc) as tc:
        with (
            tc.tile_pool(name="xp", bufs=3) as xp,
            tc.tile_pool(name="idxp", bufs=3) as idxp,
            tc.tile_pool(name="sm", bufs=1) as sm,
            tc.tile_pool(name="wk", bufs=3) as wk,
            tc.tile_pool(name="ps", bufs=3, space="PSUM") as ps,
            tc.tile_pool(name="psc", bufs=1, space="PSUM") as psc,
            tc.tile_pool(name="ps1", bufs=1, space="PSUM") as ps1,
        ):
            # a tiny dummy activation binds the ACT function-table load at
            # t~0 instead of on sample 0's critical path
            dummy_sb = sm.tile([1, 2], F32)
            nc.vector.memset(dummy_sb[:], 0.0)
            nc.scalar.activation(
                dummy_sb[:], dummy_sb[:],
                mybir.ActivationFunctionType.Sigmoid, bias=0.0, scale=1.0,
            )

            idxs = []
            xts = []

            def load_dma(s):
                xt = xp.tile([P, PF], F32, name=f"xt{s}", tag="xt")
                nc.sync.dma_start(out=xt[:], in_=x[s])
                xts.append(xt)

            # weight blob first (ACT planes gate the fill on abias), then
            # sample 0 in two halves, then x(1).
            xt0 = xp.tile([P, PF], F32, name="xt0", tag="xt")
            xts.append(xt0)
            HMM = NMM // 2
            nc.sync.dma_start(out=xt0[:, 0 : PF // 2], in_=x[0][:, 0 : PF // 2])
            blob_sb = sm.tile([P, BLOB_COLS], F32)
            nc.sync.dma_start(out=blob_sb[:], in_=wblob[:])
            nc.sync.dma_start(out=xt0[:, PF // 2 : PF], in_=x[0][:, PF // 2 : PF])
            load_dma(1)
            c0 = 0
            mask_sb = blob_sb[0:64, c0 : c0 + 64]; c0 += 64
            e8_sb = blob_sb[0:64, c0 : c0 + 8]; c0 += 8
            abias_sb = blob_sb[:, c0 : c0 + 8]; c0 += 8
            dtb_sb = blob_sb[0:8, c0 : c0 + 8]; c0 += 8
            b2_sb = blob_sb[:, c0 : c0 + 1]; c0 += 1
            w1_sb = blob_sb[0:8, c0 : c0 + 256].rearrange(
                "b (a j) -> b a j", a=8
            ); c0 += 256   # [b, a, j]
            w2t_sb = blob_sb[0:32, c0 : c0 + 128]; c0 += 128
            b1_sb = blob_sb[0:32, c0 : c0 + 1]; c0 += 1

            # double-buffered step planes; ones plane written once each
            sh_tiles = [
                sm.tile([P, NMM, 8, G], BF16, name=f"sh{i}", tag=f"sh{i}")
                for i in range(2)
            ]
            sl_tiles = [
                sm.tile([P, NMM, 8, G], BF16, name=f"sl{i}", tag=f"sl{i}")
                for i in range(2)
            ]
            for i in range(2):
                nc.gpsimd.memset(sh_tiles[i][:, :, 0, :], 1.0)
                nc.vector.memset(sl_tiles[i][:, :, 0, :], 1.0)

            # C2 for all samples: [b, (s, a)], written by one matmul/sample
            t2b_ps = ps1.tile([8, SPC, 8], F32)

            los = []

            def emit_idx(s, e0=0, e1=PF):
                if len(idxs) <= s:
                    idxs.append(idxp.tile([P, PF], I16, name=f"idx{s}", tag="idx"))
                    los.append(idxp.tile([P, PF], I16, name=f"lo{s}", tag="lo"))
                nc.vector.tensor_scalar(
                    idxs[s][:, e0:e1], xts[s][:, e0:e1], SCALE, 0.5,
                    AluOpType.mult, AluOpType.subtract,
                )
                nc.vector.tensor_scalar(
                    los[s][:, e0:e1], idxs[s][:, e0:e1], 7, None,
                    AluOpType.bitwise_and,
                )

            def emit_planes(s, j0=0, j1=NMM, h4_act=None):
                if h4_act is None:
                    h4_act = H4_ACT
                idx_v = idxs[s][:].rearrange("p (j g) -> p j g", g=G)
                lo_v = los[s][:].rearrange("p (j g) -> p j g", g=G)
                SH = sh_tiles[s % 2]
                SL = sl_tiles[s % 2]
                # hi planes (idx >= 8a) -> SH[:, :, a, :]; planes 1 and 4
                # are split by j-range across engines for exact balance
                s1 = min(max(H1_VE, j0), j1)
                if s1 > j0:
                    nc.vector.tensor_scalar(
                        SH[:, j0:s1, 1, :], idx_v[:, j0:s1, :], 8.0, None,
                        AluOpType.is_ge,
                    )
                if j1 > s1:
                    nc.gpsimd.tensor_scalar(
                        SH[:, s1:j1, 1, :], idx_v[:, s1:j1, :], 8.0, None,
                        AluOpType.is_ge,
                    )
                for a in POOL_HI:
                    nc.gpsimd.tensor_scalar(
                        SH[:, j0:j1, a, :], idx_v[:, j0:j1, :], float(8 * a),
                        None, AluOpType.is_ge,
                    )
                for a in ACT_HI:
                    # saturated sigmoid: exactly 0/1 in bf16 for integer idx
                    nc.scalar.activation(
                        SH[:, j0:j1, a, :],
                        idx_v[:, j0:j1, :],
                        mybir.ActivationFunctionType.Sigmoid,
                        bias=abias_sb[:, a : a + 1],
                        scale=64.0,
                    )
                s4 = min(max(h4_act, j0), j1)
                if s4 > j0:
                    nc.scalar.activation(
                        SH[:, j0:s4, ALT_HI, :],
                        idx_v[:, j0:s4, :],
                        mybir.ActivationFunctionType.Sigmoid,
                        bias=abias_sb[:, ALT_HI : ALT_HI + 1],
                        scale=64.0,
                    )
                if j1 > s4:
                    nc.vector.tensor_scalar(
                        SH[:, s4:j1, ALT_HI, :], idx_v[:, s4:j1, :],
                        float(8 * ALT_HI), None, AluOpType.is_ge,
                    )
                # lo planes (lo >= b), one single-op pass each
                for b in range(1, 8):
                    nc.vector.tensor_scalar(
                        SL[:, j0:j1, b, :], lo_v[:, j0:j1, :], float(b), None,
                        AluOpType.is_ge,
                    )

            cps_tiles = {}

            def emit_gram(s):
                SH = sh_tiles[s % 2]
                SL = sl_tiles[s % 2]
                c_ps = ps.tile([64, 64], F32, tag="cps")
                cps_tiles[s] = c_ps
                for j in range(NMM):
                    nc.tensor.matmul(
                        c_ps[:],
                        SL[:, j].rearrange("p b g -> p (b g)"),
                        SH[:, j].rearrange("p a g -> p (a g)"),
                        start=(j == 0),
                        stop=(j == NMM - 1),
                    )
                if CUSHION_MM:
                    cu_ps = psc.tile([64, 64], F32, tag="cush")
                    for j in range(CUSHION_MM):
                        nc.tensor.matmul(
                            cu_ps[:],
                            SL[:, j].rearrange("p b g -> p (b g)"),
                            SH[:, j].rearrange("p a g -> p (a g)"),
                            start=(j == 0),
                            stop=(j == CUSHION_MM - 1),
                        )

            def emit_epilogue(s, ladder_ve=False):
                c_ps = cps_tiles[s]
                eng = nc.vector if ladder_ve else nc.gpsimd
                # mask g != g' cross terms (VE: GpSimd cannot read PSUM),
                # then halving-add ladder over g' (idle GpSimd; VE for the
                # last sample where the Q7 launches would sit on the tail)
                cm = wk.tile([64, 8, G], F32, tag="cm")
                nc.vector.tensor_tensor(
                    cm[:].rearrange("p a g -> p (a g)"), c_ps[:], mask_sb[:],
                    AluOpType.mult,
                )
                ch4 = wk.tile([64, 8, 4], F32, tag="ch4")
                eng.tensor_tensor(
                    ch4[:], cm[:, :, 0:4], cm[:, :, 4:8], AluOpType.add
                )
                ch2 = wk.tile([64, 8, 2], F32, tag="ch2")
                eng.tensor_tensor(
                    ch2[:], ch4[:, :, 0:2], ch4[:, :, 2:4], AluOpType.add
                )
                cred = wk.tile([64, 8], F32, tag="cred")
                eng.tensor_tensor(
                    cred[:], ch2[:, :, 0], ch2[:, :, 1], AluOpType.add
                )
                # C2[b, a] for this sample straight into the shared PSUM tile
                nc.tensor.matmul(
                    t2b_ps[:, s, :], e8_sb[:], cred[:], start=True, stop=True
                )

            # emission order is engine-queue order: keep VE planes ahead of
            # the next idx, and the PE epilogue of s-1 behind Gram(s), so no
            # engine queue head ever waits on a slower producer.
            # sample 0 in two halves behind two half-DMAs (shorter fill);
            # its share of plane 4 goes to VE (the fill is ACT-gated)
            emit_idx(0, 0, PF // 2)
            emit_planes(0, 0, HMM)
            emit_idx(0, PF // 2, PF)
            emit_planes(0, HMM, NMM)
            for s in range(SPC):
                if s + 2 < SPC:
                    load_dma(s + 2)
                if s > 0:
                    emit_planes(s)
                if s + 1 < SPC:
                    emit_idx(s + 1)
                emit_gram(s)
                if s > 0:
                    emit_epilogue(s - 1)

            # ---- tail: the whole MLP runs twice, samples 0..14 overlap
            # the last Gram/epilogue; only sample 15's column rides the
            # serial late chain ----
            t2b_sb = sm.tile([8, SPC, 8], F32)
            h1_ps = ps1.tile([32, SPC], F32)
            h1r_sb = sm.tile([32, SPC], F32)
            out_ps = ps1.tile([P, SPC], F32)
            out_sb = sm.tile([P, SPC], F32)
            SL15 = SPC - 1

            def emit_mlp_tail(s0, s1):
                nc.scalar.activation(
                    t2b_sb[:, s0:s1].rearrange("p s a -> p (s a)"),
                    t2b_ps[:, s0:s1].rearrange("p s a -> p (s a)"),
                    mybir.ActivationFunctionType.Copy,
                    bias=0.0,
                    scale=1.0,
                )
                # h1 = sum_a What[:, a, :].T @ C2[:, (s, a)]; both second-
                # difference matrices and the 1/N are folded into What
                for a in range(8):
                    nc.tensor.matmul(
                        h1_ps[:, s0:s1],
                        w1_sb[:, a, :],
                        t2b_sb[:, s0:s1, a],
                        start=(a == 0),
                        stop=(a == 7),
                    )
                nc.scalar.activation(
                    h1r_sb[:, s0:s1], h1_ps[:, s0:s1],
                    mybir.ActivationFunctionType.Relu, bias=b1_sb, scale=1.0,
                )
                nc.tensor.matmul(
                    out_ps[:, s0:s1], w2t_sb, h1r_sb[:, s0:s1],
                    start=True, stop=True,
                )
                nc.scalar.activation(
                    out_sb[:, s0:s1], out_ps[:, s0:s1],
                    mybir.ActivationFunctionType.Identity, bias=b2_sb, scale=1.0,
                )
                nc.sync.dma_start(out=y[:, s0:s1], in_=out_sb[:, s0:s1])

            emit_mlp_tail(0, SL15)
            emit_epilogue(SL15, ladder_ve=True)
            emit_mlp_tail(SL15, SPC)

    nc.compile()
    return nc


_NC_CACHE = {}


def kernel(grad_map, W1, b1, W2, b2, _trace=False):
    grad_map = np.ascontiguousarray(grad_map, dtype=np.float32)
    W1 = np.asarray(W1, dtype=np.float32)
    b1 = np.asarray(b1, dtype=np.float32)
    W2 = np.asarray(W2, dtype=np.float32)
    b2 = np.asarray(b2, dtype=np.float32)

    if "nc" not in _NC_CACHE:
        _NC_CACHE["nc"] = build_kernel()
    nc = _NC_CACHE["nc"]

    blob = np.zeros((P, BLOB_COLS), np.float32)
    c0 = 0
    # mask[(b,g), (a,g')] = delta_{g,g'}
    blob[0:64, c0 : c0 + 64] = np.kron(
        np.ones((8, 8), np.float32), np.eye(G, dtype=np.float32)
    ); c0 += 64
    # e8[(b,g), b'] = delta_{b,b'}
    blob[0:64, c0 : c0 + 8] = np.kron(
        np.eye(8, dtype=np.float32), np.ones((G, 1), np.float32)
    ); c0 += 8
    # sigmoid bias per hi plane a: 64*(0.5 - 8a)
    blob[:, c0 : c0 + 8] = np.array(
        [64.0 * (0.5 - 8.0 * a) for a in range(8)], np.float32
    )[None, :]; c0 += 8
    # dtb slot kept for layout compatibility (no longer used on-device)
    dbm = np.eye(8, dtype=np.float32) - np.eye(8, k=1, dtype=np.float32)
    blob[0:8, c0 : c0 + 8] = (dbm / np.float32(NPEL)).T; c0 += 8
    blob[:, c0] = b2; c0 += 1
    # What[b, a, j] = sum_{b',a'} DB[b',b] DA[a',a] W1[j, 8a'+b'] / N:
    # h1 = sum_{b,a} What[b,a,j] C2[a,b] equals W1 @ histn (both second
    # differences of C2 folded into the weights)
    w1r = W1.T.reshape(8, 8, 32).transpose(1, 0, 2)      # [b', a', j]
    what = np.einsum("ki,lj,klm->ijm", dbm, dbm, w1r) / np.float32(NPEL)
    blob[0:8, c0 : c0 + 256] = what.reshape(8, 256).astype(np.float32); c0 += 256
    blob[0:32, c0 : c0 + 128] = W2.T; c0 += 128
    blob[0:32, c0] = b1; c0 += 1
    assert c0 == BLOB_COLS

    xs = grad_map.reshape(N_CORES, SPC, P, PF)
    in_maps = [
        {"x": np.ascontiguousarray(xs[c]), "wblob": blob} for c in range(N_CORES)
    ]

    res = run_bass_kernel_spmd(
        nc, in_maps, core_ids=list(range(N_CORES)), trace=_trace
    )
    out = np.concatenate([r["y"].T for r in res.results], axis=0)
    if _trace:
        return out, res
    return out


# revision 6
# speedup vs baseline: 1.0030x; 1.0026x over previous
"""Trainium2 Bass kernel v2: per-sample 64-bin histogram + normalize + tiny MLP.

Input  grad_map [128, 512, 512] f32, W1 [32,64], b1 [32], W2 [128,32], b2 [128]
Output [128, 128] f32 = relu(hist_norm @ W1.T + b1) @ W2.T + b2
Sharding: pure data parallel over batch across 8 cores (16 samples/core).

Strategy (64 bins = 8 hi x 8 lo, joint counts by a TensorE Gram):
- idx = round(x*64/255 - 0.5) i16 (1 VE pass at 2x DVE rate)
- 7 hi step planes (idx >= 8a) and 7 lo planes ((idx&7) >= b, fused
  and+cmp in ONE tensor_scalar) + 1 ones plane (memset once per buffer)
- planes split VE 8.5 / ACT 3.5 / Pool 2 by engine-rate balance; the
  ACT planes use saturated Sigmoid(64*(idx-8a+0.5)) which is exactly
  0/1 in bf16, so no sign-correction anywhere
- Gram: C[(b,g),(a,g')] += SL[:,j].T @ SH[:,j] over 128 f-interleaved
  chunks; mask out g!=g', halving-add ladder over g' (idle Pool), then
  e8.T @ cred -> C2[b, a] per sample written into a shared PSUM tile
- 2nd difference of C2 + MLP tail, all b-major

Scheduling (the critical part, engine queues run in emission order):
- per iteration: dma(s+2); planes(s); idx(s+1); Gram(s); epilogue(s-1)
  so no engine queue head ever waits on a slower producer
- PE work/sample (6.87us) slightly exceeds every plane engine, keeping
  the PE continuously busy at full p-state (it is the pipeline pacer)
- all weights ride ONE blob DMA, dispatched after x(0)/x(1)
"""

import numpy as np

import concourse.bacc as bacc
import concourse.mybir as mybir
from concourse.mybir import AluOpType
from concourse.tile import TileContext
from concourse.bass_utils import run_bass_kernel_spmd

HIST_BINS = 64
VMAX = 255.0
SCALE = float(np.float32(HIST_BINS / VMAX))
B, H, W = 128, 512, 512
N_CORES = 8
SPC = B // N_CORES            # 16 samples per core
NPEL = H * W                  # 262144
P = 128
PF = NPEL // P                # 2048 free elems per partition
G = 8                         # f-columns per Gram matmul
NMM = PF // G                 # 128 matmuls per sample

F32 = mybir.dt.float32
I16 = mybir.dt.int16
BF16 = mybir.dt.bfloat16

POOL_HI = (2, 3)              # hi planes always built on GpSimd
ACT_HI = (5, 6, 7)            # hi planes always built on ScalarE
ALT_HI = 4                    # hi plane split ACT/VE by j-range
H1_VE = 236                   # VE builds j 0:H1_VE of hi plane 1, Pool rest
H4_ACT = 158                  # ACT builds j 0:H4_ACT of hi plane 4, VE rest
CUSHION_MM = 0                # extra PE matmuls/sample so the PE stays the
                              # (p-state-hot) pipeline pacer

# weight blob column layout: [mask 128 | e8 8 | abias 8 | dtb 8 | b2 1 | rest]
# rows 0..63 of 'rest' hold W1.T ([8a+b, j] 32 cols), rows 64..95 hold W2.T
# ([j, 128] needs 128 cols) - W2.T gets its own region instead.
BLOB_COLS = 64 + 8 + 8 + 8 + 1 + 256 + 128 + 1  # mask e8 abias dtb b2 w1 w2t b1


def build_kernel():
    nc = bacc.Bacc("TRN2", target_bir_lowering=False)

    x = nc.dram_tensor("x", [SPC, P, PF], F32, kind="ExternalInput")
    wblob = nc.dram_tensor("wblob", [P, BLOB_COLS], F32, kind="ExternalInput")
    y = nc.dram_tensor("y", [P, SPC], F32, kind="ExternalOutput")

    with TileContext(nc) as tc:
        with (
            tc.tile_pool(name="xp", bufs=3) as xp,
            tc.tile_pool(name="idxp", bufs=3) as idxp,
            tc.tile_pool(name="sm", bufs=1) as sm,
            tc.tile_pool(name="wk", bufs=3) as wk,
            tc.tile_pool(name="ps", bufs=3, space="PSUM") as ps,
            tc.tile_pool(name="psc", bufs=1, space="PSUM") as psc,
            tc.tile_pool(name="ps1", bufs=1, space="PSUM") as ps1,
        ):
            # a tiny dummy activation binds the ACT function-table load at
            # t~0 instead of on sample 0's critical path
            dummy_sb = sm.tile([1, 2], F32)
            nc.vector.memset(dummy_sb[:], 0.0)
            nc.scalar.activation(
                dummy_sb[:], dummy_sb[:],
                mybir.ActivationFunctionType.Sigmoid, bias=0.0, scale=1.0,
            )

            idxs = []
            xts = []

            def load_dma(s):
                xt = xp.tile([P, PF], F32, name=f"xt{s}", tag="xt")
                nc.sync.dma_start(out=xt[:], in_=x[s])
                xts.append(xt)

            # weight blob first (ACT planes gate the fill on abias), then
            # sample 0 in two halves, then x(1).
            xt0 = xp.tile([P, PF], F32, name="xt0", tag="xt")
            xts.append(xt0)
            HMM = NMM // 2
            nc.sync.dma_start(out=xt0[:, 0 : PF // 2], in_=x[0][:, 0 : PF // 2])
            blob_sb = sm.tile([P, BLOB_COLS], F32)
            nc.sync.dma_start(out=blob_sb[:], in_=wblob[:])
            nc.sync.dma_start(out=xt0[:, PF // 2 : PF], in_=x[0][:, PF // 2 : PF])
            load_dma(1)
            c0 = 0
            mask_sb = blob_sb[0:64, c0 : c0 + 64]; c0 += 64
            e8_sb = blob_sb[0:64, c0 : c0 + 8]; c0 += 8
            abias_sb = blob_sb[:, c0 : c0 + 8]; c0 += 8
            dtb_sb = blob_sb[0:8, c0 : c0 + 8]; c0 += 8
            b2_sb = blob_sb[:, c0 : c0 + 1]; c0 += 1
            w1_sb = blob_sb[0:8, c0 : c0 + 256].rearrange(
                "b (a j) -> b a j", a=8
            ); c0 += 256   # [b, a, j]
            w2t_sb = blob_sb[0:32, c0 : c0 + 128]; c0 += 128
            b1_sb = blob_sb[0:32, c0 : c0 + 1]; c0 += 1

            # double-buffered step planes; ones plane written once each
            sh_tiles = [
                sm.tile([P, NMM, 8, G], BF16, name=f"sh{i}", tag=f"sh{i}")
                for i in range(2)
            ]
            sl_tiles = [
                sm.tile([P, NMM, 8, G], BF16, name=f"sl{i}", tag=f"sl{i}")
                for i in range(2)
            ]
            for i in range(2):
                nc.gpsimd.memset(sh_tiles[i][:, :, 0, :], 1.0)
                nc.vector.memset(sl_tiles[i][:, :, 0, :], 1.0)

            # C2 for all samples: [b, (s, a)], written by one matmul/sample
            t2b_ps = ps1.tile([8, SPC, 8], F32)

            los = []

            def emit_idx(s, e0=0, e1=PF):
                if len(idxs) <= s:
                    idxs.append(idxp.tile([P, PF], I16, name=f"idx{s}", tag="idx"))
                    los.append(idxp.tile([P, PF], I16, name=f"lo{s}", tag="lo"))
                nc.vector.tensor_scalar(
                    idxs[s][:, e0:e1], xts[s][:, e0:e1], SCALE, 0.5,
                    AluOpType.mult, AluOpType.subtract,
                )
                nc.vector.tensor_scalar(
                    los[s][:, e0:e1], idxs[s][:, e0:e1], 7, None,
                    AluOpType.bitwise_and,
                )

            def emit_planes(s, j0=0, j1=NMM, h4_act=None):
                if h4_act is None:
                    h4_act = H4_ACT
                idx_v = idxs[s][:].rearrange("p (j g) -> p j g", g=G)
                lo_v = los[s][:].rearrange("p (j g) -> p j g", g=G)
                SH = sh_tiles[s % 2]
                SL = sl_tiles[s % 2]
                # hi planes (idx >= 8a) -> SH[:, :, a, :]; planes 1 and 4
                # are split by j-range across engines for exact balance
                s1 = min(max(H1_VE, j0), j1)
                if s1 > j0:
                    nc.vector.tensor_scalar(
                        SH[:, j0:s1, 1, :], idx_v[:, j0:s1, :], 8.0, None,
                        AluOpType.is_ge,
                    )
                if j1 > s1:
                    nc.gpsimd.tensor_scalar(
                        SH[:, s1:j1, 1, :], idx_v[:, s1:j1, :], 8.0, None,
                        AluOpType.is_ge,
                    )
                for a in POOL_HI:
                    nc.gpsimd.tensor_scalar(
                        SH[:, j0:j1, a, :], idx_v[:, j0:j1, :], float(8 * a),
                        None, AluOpType.is_ge,
                    )
                for a in ACT_HI:
                    # saturated sigmoid: exactly 0/1 in bf16 for integer idx
                    nc.scalar.activation(
                        SH[:, j0:j1, a, :],
                        idx_v[:, j0:j1, :],
                        mybir.ActivationFunctionType.Sigmoid,
                        bias=abias_sb[:, a : a + 1],
                        scale=64.0,
                    )
                s4 = min(max(h4_act, j0), j1)
                if s4 > j0:
                    nc.scalar.activation(
                        SH[:, j0:s4, ALT_HI, :],
                        idx_v[:, j0:s4, :],
                        mybir.ActivationFunctionType.Sigmoid,
                        bias=abias_sb[:, ALT_HI : ALT_HI + 1],
                        scale=64.0,
                    )
                if j1 > s4:
                    nc.vector.tensor_scalar(
                        SH[:, s4:j1, ALT_HI, :], idx_v[:, s4:j1, :],
                        float(8 * ALT_HI), None, AluOpType.is_ge,
                    )
                # lo planes (lo >= b), one single-op pass each
                for b in range(1, 8):
                    nc.vector.tensor_scalar(
                        SL[:, j0:j1, b, :], lo_v[:, j0:j1, :], float(b), None,
                        AluOpType.is_ge,
                    )

            cps_tiles = {}

            def emit_gram(s):
                SH = sh_tiles[s % 2]
                SL = sl_tiles[s % 2]
                c_ps = ps.tile([64, 64], F32, tag="cps")
                cps_tiles[s] = c_ps
                for j in range(NMM):
                    nc.tensor.matmul(
                        c_ps[:],
                        SL[:, j].rearrange("p b g -> p (b g)"),
                        SH[:, j].rearrange("p a g -> p (a g)"),
                        start=(j == 0),
                        stop=(j == NMM - 1),
                    )
                if CUSHION_MM:
                    cu_ps = psc.tile([64, 64], F32, tag="cush")
                    for j in range(CUSHION_MM):
                        nc.tensor.matmul(
                            cu_ps[:],
                            SL[:, j].rearrange("p b g -> p (b g)"),
                            SH[:, j].rearrange("p a g -> p (a g)"),
                            start=(j == 0),
                            stop=(j == CUSHION_MM - 1),
                        )

            def emit_epilogue(s, ladder_ve=False):
                c_ps = cps_tiles[s]
                eng = nc.vector if ladder_ve else nc.gpsimd
                # mask g != g' cross terms (VE: GpSimd cannot read PSUM),
                # then halving-add ladder over g' (idle GpSimd; VE for the
                # last sample where the Q7 launches would sit on the tail)
                cm = wk.tile([64, 8, G], F32, tag="cm")
                nc.vector.tensor_tensor(
                    cm[:].rearrange("p a g -> p (a g)"), c_ps[:], mask_sb[:],
                    AluOpType.mult,
                )
                ch4 = wk.tile([64, 8, 4], F32, tag="ch4")
                eng.tensor_tensor(
                    ch4[:], cm[:, :, 0:4], cm[:, :, 4:8], AluOpType.add
                )
                ch2 = wk.tile([64, 8, 2], F32, tag="ch2")
                eng.tensor_tensor(
                    ch2[:], ch4[:, :, 0:2], ch4[:, :, 2:4], AluOpType.add
                )
                cred = wk.tile([64, 8], F32, tag="cred")
                eng.tensor_tensor(
                    cred[:], ch2[:, :, 0], ch2[:, :, 1], AluOpType.add
                )
                # C2[b, a] for this sample straight into the shared PSUM tile
                nc.tensor.matmul(
                    t2b_ps[:, s, :], e8_sb[:], cred[:], start=True, stop=True
                )

            # emission order is engine-queue order: keep VE planes ahead of
            # the next idx, and the PE epilogue of s-1 behind Gram(s), so no
            # engine queue head ever waits on a slower producer.
            # sample 0 in two halves behind two half-DMAs (shorter fill);
            # its share of plane 4 goes to VE (the fill is ACT-gated)
            emit_idx(0, 0, PF // 2)
            emit_planes(0, 0, HMM)
            emit_idx(0, PF // 2, PF)
            emit_planes(0, HMM, NMM)
            for s in range(SPC):
                if s + 2 < SPC:
                    load_dma(s + 2)
                if s > 0:
                    emit_planes(s)
                if s + 1 < SPC:
                    emit_idx(s + 1)
                emit_gram(s)
                if s > 0:
                    emit_epilogue(s - 1)

            # ---- tail: the whole MLP runs twice, samples 0..14 overlap
            # the last Gram/epilogue; only sample 15's column rides the
            # serial late chain ----
            t2b_sb = sm.tile([8, SPC, 8], F32)
            h1_ps = ps1.tile([32, SPC], F32)
            h1r_sb = sm.tile([32, SPC], F32)
            out_ps = ps1.tile([P, SPC], F32)
            out_sb = sm.tile([P, SPC], F32)
            SL15 = SPC - 1

            def emit_mlp_tail(s0, s1):
                nc.scalar.activation(
                    t2b_sb[:, s0:s1].rearrange("p s a -> p (s a)"),
                    t2b_ps[:, s0:s1].rearrange("p s a -> p (s a)"),
                    mybir.ActivationFunctionType.Copy,
                    bias=0.0,
                    scale=1.0,
                )
                # h1 = sum_a What[:, a, :].T @ C2[:, (s, a)]; both second-
                # difference matrices and the 1/N are folded into What
                for a in range(8):
                    nc.tensor.matmul(
                        h1_ps[:, s0:s1],
                        w1_sb[:, a, :],
                        t2b_sb[:, s0:s1, a],
                        start=(a == 0),
                        stop=(a == 7),
                    )
                nc.scalar.activation(
                    h1r_sb[:, s0:s1], h1_ps[:, s0:s1],
                    mybir.ActivationFunctionType.Relu, bias=b1_sb, scale=1.0,
                )
                nc.tensor.matmul(
                    out_ps[:, s0:s1], w2t_sb, h1r_sb[:, s0:s1],
                    start=True, stop=True,
                )
                nc.scalar.activation(
                    out_sb[:, s0:s1], out_ps[:, s0:s1],
                    mybir.ActivationFunctionType.Identity, bias=b2_sb, scale=1.0,
                )
                nc.sync.dma_start(out=y[:, s0:s1], in_=out_sb[:, s0:s1])

            emit_mlp_tail(0, SL15)
            emit_epilogue(SL15, ladder_ve=True)
            emit_mlp_tail(SL15, SPC)

    nc.compile()
    return nc


_NC_CACHE = {}


def kernel(grad_map, W1, b1, W2, b2, _trace=False):
    grad_map = np.ascontiguousarray(grad_map, dtype=np.float32)
    W1 = np.asarray(W1, dtype=np.float32)
    b1 = np.asarray(b1, dtype=np.float32)
    W2 = np.asarray(W2, dtype=np.float32)
    b2 = np.asarray(b2, dtype=np.float32)

    if "nc" not in _NC_CACHE:
        _NC_CACHE["nc"] = build_kernel()
    nc = _NC_CACHE["nc"]

    blob = np.zeros((P, BLOB_COLS), np.float32)
    c0 = 0
    # mask[(b,g), (a,g')] = delta_{g,g'}
    blob[0:64, c0 : c0 + 64] = np.kron(
        np.ones((8, 8), np.float32), np.eye(G, dtype=np.float32)
    ); c0 += 64
    # e8[(b,g), b'] = delta_{b,b'}
    blob[0:64, c0 : c0 + 8] = np.kron(
        np.eye(8, dtype=np.float32), np.ones((G, 1), np.float32)
    ); c0 += 8
    # sigmoid bias per hi plane a: 64*(0.5 - 8a)
    blob[:, c0 : c0 + 8] = np.array(
        [64.0 * (0.5 - 8.0 * a) for a in range(8)], np.float32
    )[None, :]; c0 += 8
    # dtb slot kept for layout compatibility (no longer used on-device)
    dbm = np.eye(8, dtype=np.float32) - np.eye(8, k=1, dtype=np.float32)
    blob[0:8, c0 : c0 + 8] = (dbm / np.float32(NPEL)).T; c0 += 8
    blob[:, c0] = b2; c0 += 1
    # What[b, a, j] = sum_{b',a'} DB[b',b] DA[a',a] W1[j, 8a'+b'] / N:
    # h1 = sum_{b,a} What[b,a,j] C2[a,b] equals W1 @ histn (both second
    # differences of C2 folded into the weights)
    w1r = W1.T.reshape(8, 8, 32).transpose(1, 0, 2)      # [b', a', j]
    what = np.einsum("ki,lj,klm->ijm", dbm, dbm, w1r) / np.float32(NPEL)
    blob[0:8, c0 : c0 + 256] = what.reshape(8, 256).astype(np.float32); c0 += 256
    blob[0:32, c0 : c0 + 128] = W2.T; c0 += 128
    blob[0:32, c0] = b1; c0 += 1
    assert c0 == BLOB_COLS

    xs = grad_map.reshape(N_CORES, SPC, P, PF)
    in_maps = [
        {"x": np.ascontiguousarray(xs[c]), "wblob": blob} for c in range(N_CORES)
    ]

    res = run_bass_kernel_spmd(
        nc, in_maps, core_ids=list(range(N_CORES)), trace=_trace
    )
    out = np.concatenate([r["y"].T for r in res.results], axis=0)
    if _trace:
        return out, res
    return out


# revision 7
# speedup vs baseline: 1.0034x; 1.0004x over previous
"""Trainium2 Bass kernel v2: per-sample 64-bin histogram + normalize + tiny MLP.

Input  grad_map [128, 512, 512] f32, W1 [32,64], b1 [32], W2 [128,32], b2 [128]
Output [128, 128] f32 = relu(hist_norm @ W1.T + b1) @ W2.T + b2
Sharding: pure data parallel over batch across 8 cores (16 samples/core).

Strategy (64 bins = 8 hi x 8 lo, joint counts by a TensorE Gram):
- idx = round(x*64/255 - 0.5) i16 (1 VE pass at 2x DVE rate)
- 7 hi step planes (idx >= 8a) and 7 lo planes ((idx&7) >= b, fused
  and+cmp in ONE tensor_scalar) + 1 ones plane (memset once per buffer)
- planes split VE 8.5 / ACT 3.5 / Pool 2 by engine-rate balance; the
  ACT planes use saturated Sigmoid(64*(idx-8a+0.5)) which is exactly
  0/1 in bf16, so no sign-correction anywhere
- Gram: C[(b,g),(a,g')] += SL[:,j].T @ SH[:,j] over 128 f-interleaved
  chunks; mask out g!=g', halving-add ladder over g' (idle Pool), then
  e8.T @ cred -> C2[b, a] per sample written into a shared PSUM tile
- 2nd difference of C2 + MLP tail, all b-major

Scheduling (the critical part, engine queues run in emission order):
- per iteration: dma(s+2); planes(s); idx(s+1); Gram(s); epilogue(s-1)
  so no engine queue head ever waits on a slower producer
- PE work/sample (6.87us) slightly exceeds every plane engine, keeping
  the PE continuously busy at full p-state (it is the pipeline pacer)
- all weights ride ONE blob DMA, dispatched after x(0)/x(1)
"""

import numpy as np

import concourse.bacc as bacc
import concourse.mybir as mybir
from concourse.mybir import AluOpType
from concourse.tile import TileContext
from concourse.bass_utils import run_bass_kernel_spmd

HIST_BINS = 64
VMAX = 255.0
SCALE = float(np.float32(HIST_BINS / VMAX))
B, H, W = 128, 512, 512
N_CORES = 8
SPC = B // N_CORES            # 16 samples per core
NPEL = H * W                  # 262144
P = 128
PF = NPEL // P                # 2048 free elems per partition
G = 8                         # f-columns per Gram matmul
NMM = PF // G                 # 128 matmuls per sample

F32 = mybir.dt.float32
I16 = mybir.dt.int16
BF16 = mybir.dt.bfloat16

POOL_HI = (2, 3)              # hi planes always built on GpSimd
ACT_HI = (5, 6, 7)            # hi planes always built on ScalarE
ALT_HI = 4                    # hi plane split ACT/VE by j-range
H1_VE = 236                   # VE builds j 0:H1_VE of hi plane 1, Pool rest
H4_ACT = 158                  # ACT builds j 0:H4_ACT of hi plane 4, VE rest
CUSHION_MM = 0                # extra PE matmuls/sample so the PE stays the
                              # (p-state-hot) pipeline pacer

# weight blob column layout: [mask 128 | e8 8 | abias 8 | dtb 8 | b2 1 | rest]
# rows 0..63 of 'rest' hold W1.T ([8a+b, j] 32 cols), rows 64..95 hold W2.T
# ([j, 128] needs 128 cols) - W2.T gets its own region instead.
BLOB_COLS = 64 + 8 + 8 + 8 + 1 + 256 + 128 + 1  # mask e8 abias dtb b2 w1 w2t b1


def build_kernel():
    nc = bacc.Bacc("TRN2", target_bir_lowering=False)

    x = nc.dram_tensor("x", [SPC, P, PF], F32, kind="ExternalInput")
    wblob = nc.dram_tensor("wblob", [P, BLOB_COLS], F32, kind="ExternalInput")
    y = nc.dram_tensor("y", [P, SPC], F32, kind="ExternalOutput")

    with TileContext(nc) as tc:
        with (
            tc.tile_pool(name="xp", bufs=3) as xp,
            tc.tile_pool(name="idxp", bufs=3) as idxp,
            tc.tile_pool(name="sm", bufs=1) as sm,
            tc.tile_pool(name="wk", bufs=3) as wk,
            tc.tile_pool(name="ps", bufs=3, space="PSUM") as ps,
            tc.tile_pool(name="psc", bufs=1, space="PSUM") as psc,
            tc.tile_pool(name="ps1", bufs=1, space="PSUM") as ps1,
        ):
            # a tiny dummy activation binds the ACT function-table load at
            # t~0 instead of on sample 0's critical path
            dummy_sb = sm.tile([1, 2], F32)
            nc.vector.memset(dummy_sb[:], 0.0)
            nc.scalar.activation(
                dummy_sb[:], dummy_sb[:],
                mybir.ActivationFunctionType.Sigmoid, bias=0.0, scale=1.0,
            )

            idxs = []
            xts = []

            def load_dma(s):
                xt = xp.tile([P, PF], F32, name=f"xt{s}", tag="xt")
                nc.sync.dma_start(out=xt[:], in_=x[s])
                xts.append(xt)

            # weight blob first (ACT planes gate the fill on abias), then
            # sample 0 in two halves, then x(1).
            # only the fill-critical weight columns (mask/e8/abias) load
            # ahead of sample 0; the big W1/W2 half rides later (tail-only)
            blob_sb = sm.tile([P, BLOB_COLS], F32)
            BSPLIT = 64 + 8 + 8
            nc.sync.dma_start(out=blob_sb[:, 0:BSPLIT], in_=wblob[:, 0:BSPLIT])
            xt0 = xp.tile([P, PF], F32, name="xt0", tag="xt")
            xts.append(xt0)
            HMM = NMM // 2
            nc.sync.dma_start(out=xt0[:, 0 : PF // 2], in_=x[0][:, 0 : PF // 2])
            nc.sync.dma_start(out=xt0[:, PF // 2 : PF], in_=x[0][:, PF // 2 : PF])
            load_dma(1)
            nc.sync.dma_start(
                out=blob_sb[:, BSPLIT:BLOB_COLS], in_=wblob[:, BSPLIT:BLOB_COLS]
            )
            c0 = 0
            mask_sb = blob_sb[0:64, c0 : c0 + 64]; c0 += 64
            e8_sb = blob_sb[0:64, c0 : c0 + 8]; c0 += 8
            abias_sb = blob_sb[:, c0 : c0 + 8]; c0 += 8
            dtb_sb = blob_sb[0:8, c0 : c0 + 8]; c0 += 8
            b2_sb = blob_sb[:, c0 : c0 + 1]; c0 += 1
            w1_sb = blob_sb[0:8, c0 : c0 + 256].rearrange(
                "b (a j) -> b a j", a=8
            ); c0 += 256   # [b, a, j]
            w2t_sb = blob_sb[0:32, c0 : c0 + 128]; c0 += 128
            b1_sb = blob_sb[0:32, c0 : c0 + 1]; c0 += 1

            # double-buffered step planes; ones plane written once each
            sh_tiles = [
                sm.tile([P, NMM, 8, G], BF16, name=f"sh{i}", tag=f"sh{i}")
                for i in range(2)
            ]
            sl_tiles = [
                sm.tile([P, NMM, 8, G], BF16, name=f"sl{i}", tag=f"sl{i}")
                for i in range(2)
            ]
            for i in range(2):
                nc.gpsimd.memset(sh_tiles[i][:, :, 0, :], 1.0)
                nc.vector.memset(sl_tiles[i][:, :, 0, :], 1.0)

            # C2 for all samples: [b, (s, a)], written by one matmul/sample
            t2b_ps = ps1.tile([8, SPC, 8], F32)

            los = []

            def emit_idx(s, e0=0, e1=PF):
                if len(idxs) <= s:
                    idxs.append(idxp.tile([P, PF], I16, name=f"idx{s}", tag="idx"))
                    los.append(idxp.tile([P, PF], I16, name=f"lo{s}", tag="lo"))
                nc.vector.tensor_scalar(
                    idxs[s][:, e0:e1], xts[s][:, e0:e1], SCALE, 0.5,
                    AluOpType.mult, AluOpType.subtract,
                )
                nc.vector.tensor_scalar(
                    los[s][:, e0:e1], idxs[s][:, e0:e1], 7, None,
                    AluOpType.bitwise_and,
                )

            def emit_planes(s, j0=0, j1=NMM, h4_act=None):
                if h4_act is None:
                    h4_act = H4_ACT
                idx_v = idxs[s][:].rearrange("p (j g) -> p j g", g=G)
                lo_v = los[s][:].rearrange("p (j g) -> p j g", g=G)
                SH = sh_tiles[s % 2]
                SL = sl_tiles[s % 2]
                # hi planes (idx >= 8a) -> SH[:, :, a, :]; planes 1 and 4
                # are split by j-range across engines for exact balance
                s1 = min(max(H1_VE, j0), j1)
                if s1 > j0:
                    nc.vector.tensor_scalar(
                        SH[:, j0:s1, 1, :], idx_v[:, j0:s1, :], 8.0, None,
                        AluOpType.is_ge,
                    )
                if j1 > s1:
                    nc.gpsimd.tensor_scalar(
                        SH[:, s1:j1, 1, :], idx_v[:, s1:j1, :], 8.0, None,
                        AluOpType.is_ge,
                    )
                for a in POOL_HI:
                    nc.gpsimd.tensor_scalar(
                        SH[:, j0:j1, a, :], idx_v[:, j0:j1, :], float(8 * a),
                        None, AluOpType.is_ge,
                    )
                for a in ACT_HI:
                    # saturated sigmoid: exactly 0/1 in bf16 for integer idx
                    nc.scalar.activation(
                        SH[:, j0:j1, a, :],
                        idx_v[:, j0:j1, :],
                        mybir.ActivationFunctionType.Sigmoid,
                        bias=abias_sb[:, a : a + 1],
                        scale=64.0,
                    )
                s4 = min(max(h4_act, j0), j1)
                if s4 > j0:
                    nc.scalar.activation(
                        SH[:, j0:s4, ALT_HI, :],
                        idx_v[:, j0:s4, :],
                        mybir.ActivationFunctionType.Sigmoid,
                        bias=abias_sb[:, ALT_HI : ALT_HI + 1],
                        scale=64.0,
                    )
                if j1 > s4:
                    nc.vector.tensor_scalar(
                        SH[:, s4:j1, ALT_HI, :], idx_v[:, s4:j1, :],
                        float(8 * ALT_HI), None, AluOpType.is_ge,
                    )
                # lo planes (lo >= b), one single-op pass each
                for b in range(1, 8):
                    nc.vector.tensor_scalar(
                        SL[:, j0:j1, b, :], lo_v[:, j0:j1, :], float(b), None,
                        AluOpType.is_ge,
                    )

            cps_tiles = {}

            def emit_gram(s):
                SH = sh_tiles[s % 2]
                SL = sl_tiles[s % 2]
                c_ps = ps.tile([64, 64], F32, tag="cps")
                cps_tiles[s] = c_ps
                for j in range(NMM):
                    nc.tensor.matmul(
                        c_ps[:],
                        SL[:, j].rearrange("p b g -> p (b g)"),
                        SH[:, j].rearrange("p a g -> p (a g)"),
                        start=(j == 0),
                        stop=(j == NMM - 1),
                    )
                if CUSHION_MM:
                    cu_ps = psc.tile([64, 64], F32, tag="cush")
                    for j in range(CUSHION_MM):
                        nc.tensor.matmul(
                            cu_ps[:],
                            SL[:, j].rearrange("p b g -> p (b g)"),
                            SH[:, j].rearrange("p a g -> p (a g)"),
                            start=(j == 0),
                            stop=(j == CUSHION_MM - 1),
                        )

            def emit_epilogue(s, ladder_ve=False):
                c_ps = cps_tiles[s]
                eng = nc.vector if ladder_ve else nc.gpsimd
                # mask g != g' cross terms (VE: GpSimd cannot read PSUM),
                # then halving-add ladder over g' (idle GpSimd; VE for the
                # last sample where the Q7 launches would sit on the tail)
                cm = wk.tile([64, 8, G], F32, tag="cm")
                nc.vector.tensor_tensor(
                    cm[:].rearrange("p a g -> p (a g)"), c_ps[:], mask_sb[:],
                    AluOpType.mult,
                )
                ch4 = wk.tile([64, 8, 4], F32, tag="ch4")
                eng.tensor_tensor(
                    ch4[:], cm[:, :, 0:4], cm[:, :, 4:8], AluOpType.add
                )
                ch2 = wk.tile([64, 8, 2], F32, tag="ch2")
                eng.tensor_tensor(
                    ch2[:], ch4[:, :, 0:2], ch4[:, :, 2:4], AluOpType.add
                )
                cred = wk.tile([64, 8], F32, tag="cred")
                eng.tensor_tensor(
                    cred[:], ch2[:, :, 0], ch2[:, :, 1], AluOpType.add
                )
                # C2[b, a] for this sample straight into the shared PSUM tile
                nc.tensor.matmul(
                    t2b_ps[:, s, :], e8_sb[:], cred[:], start=True, stop=True
                )

            # emission order is engine-queue order: keep VE planes ahead of
            # the next idx, and the PE epilogue of s-1 behind Gram(s), so no
            # engine queue head ever waits on a slower producer.
            # sample 0 in two halves behind two half-DMAs (shorter fill);
            # its share of plane 4 goes to VE (the fill is ACT-gated)
            emit_idx(0, 0, PF // 2)
            emit_planes(0, 0, HMM)
            emit_idx(0, PF // 2, PF)
            emit_planes(0, HMM, NMM)
            for s in range(SPC):
                if s + 2 < SPC:
                    load_dma(s + 2)
                if s > 0:
                    emit_planes(s)
                if s + 1 < SPC:
                    emit_idx(s + 1)
                emit_gram(s)
                if s > 0:
                    emit_epilogue(s - 1)

            # ---- tail: the whole MLP runs twice, samples 0..14 overlap
            # the last Gram/epilogue; only sample 15's column rides the
            # serial late chain ----
            t2b_sb = sm.tile([8, SPC, 8], F32)
            h1_ps = ps1.tile([32, SPC], F32)
            h1r_sb = sm.tile([32, SPC], F32)
            out_ps = ps1.tile([P, SPC], F32)
            out_sb = sm.tile([P, SPC], F32)
            SL15 = SPC - 1

            def emit_mlp_tail(s0, s1):
                nc.scalar.activation(
                    t2b_sb[:, s0:s1].rearrange("p s a -> p (s a)"),
                    t2b_ps[:, s0:s1].rearrange("p s a -> p (s a)"),
                    mybir.ActivationFunctionType.Copy,
                    bias=0.0,
                    scale=1.0,
                )
                # h1 = sum_a What[:, a, :].T @ C2[:, (s, a)]; both second-
                # difference matrices and the 1/N are folded into What
                for a in range(8):
                    nc.tensor.matmul(
                        h1_ps[:, s0:s1],
                        w1_sb[:, a, :],
                        t2b_sb[:, s0:s1, a],
                        start=(a == 0),
                        stop=(a == 7),
                    )
                nc.scalar.activation(
                    h1r_sb[:, s0:s1], h1_ps[:, s0:s1],
                    mybir.ActivationFunctionType.Relu, bias=b1_sb, scale=1.0,
                )
                nc.tensor.matmul(
                    out_ps[:, s0:s1], w2t_sb, h1r_sb[:, s0:s1],
                    start=True, stop=True,
                )
                nc.scalar.activation(
                    out_sb[:, s0:s1], out_ps[:, s0:s1],
                    mybir.ActivationFunctionType.Identity, bias=b2_sb, scale=1.0,
                )
                nc.sync.dma_start(out=y[:, s0:s1], in_=out_sb[:, s0:s1])

            emit_mlp_tail(0, SL15)
            emit_epilogue(SL15, ladder_ve=True)
            emit_mlp_tail(SL15, SPC)

    nc.compile()
    return nc


_NC_CACHE = {}


def kernel(grad_map, W1, b1, W2, b2, _trace=False):
    grad_map = np.ascontiguousarray(grad_map, dtype=np.float32)
    W1 = np.asarray(W1, dtype=np.float32)
    b1 = np.asarray(b1, dtype=np.float32)
    W2 = np.asarray(W2, dtype=np.float32)
    b2 = np.asarray(b2, dtype=np.float32)

    if "nc" not in _NC_CACHE:
        _NC_CACHE["nc"] = build_kernel()
    nc = _NC_CACHE["nc"]

    blob = np.zeros((P, BLOB_COLS), np.float32)
    c0 = 0
    # mask[(b,g), (a,g')] = delta_{g,g'}
    blob[0:64, c0 : c0 + 64] = np.kron(
        np.ones((8, 8), np.float32), np.eye(G, dtype=np.float32)
    ); c0 += 64
    # e8[(b,g), b'] = delta_{b,b'}
    blob[0:64, c0 : c0 + 8] = np.kron(
        np.eye(8, dtype=np.float32), np.ones((G, 1), np.float32)
    ); c0 += 8
    # sigmoid bias per hi plane a: 64*(0.5 - 8a)
    blob[:, c0 : c0 + 8] = np.array(
        [64.0 * (0.5 - 8.0 * a) for a in range(8)], np.float32
    )[None, :]; c0 += 8
    # dtb slot kept for layout compatibility (no longer used on-device)
    dbm = np.eye(8, dtype=np.float32) - np.eye(8, k=1, dtype=np.float32)
    blob[0:8, c0 : c0 + 8] = (dbm / np.float32(NPEL)).T; c0 += 8
    blob[:, c0] = b2; c0 += 1
    # What[b, a, j] = sum_{b',a'} DB[b',b] DA[a',a] W1[j, 8a'+b'] / N:
    # h1 = sum_{b,a} What[b,a,j] C2[a,b] equals W1 @ histn (both second
    # differences of C2 folded into the weights)
    w1r = W1.T.reshape(8, 8, 32).transpose(1, 0, 2)      # [b', a', j]
    what = np.einsum("ki,lj,klm->ijm", dbm, dbm, w1r) / np.float32(NPEL)
    blob[0:8, c0 : c0 + 256] = what.reshape(8, 256).astype(np.float32); c0 += 256
    blob[0:32, c0 : c0 + 128] = W2.T; c0 += 128
    blob[0:32, c0] = b1; c0 += 1
    assert c0 == BLOB_COLS

    xs = grad_map.reshape(N_CORES, SPC, P, PF)
    in_maps = [
        {"x": np.ascontiguousarray(xs[c]), "wblob": blob} for c in range(N_CORES)
    ]

    res = run_bass_kernel_spmd(
        nc, in_maps, core_ids=list(range(N_CORES)), trace=_trace
    )
    out = np.concatenate([r["y"].T for r in res.results], axis=0)
    if _trace:
        return out, res
    return out


# revision 9
# speedup vs baseline: 1.0049x; 1.0015x over previous
"""Trainium2 Bass kernel v2: per-sample 64-bin histogram + normalize + tiny MLP.

Input  grad_map [128, 512, 512] f32, W1 [32,64], b1 [32], W2 [128,32], b2 [128]
Output [128, 128] f32 = relu(hist_norm @ W1.T + b1) @ W2.T + b2
Sharding: pure data parallel over batch across 8 cores (16 samples/core).

Strategy (64 bins = 8 hi x 8 lo, joint counts by a TensorE Gram):
- idx = round(x*64/255 - 0.5) i16 (1 VE pass at 2x DVE rate)
- 7 hi step planes (idx >= 8a) and 7 lo planes ((idx&7) >= b, fused
  and+cmp in ONE tensor_scalar) + 1 ones plane (memset once per buffer)
- planes split VE 8.5 / ACT 3.5 / Pool 2 by engine-rate balance; the
  ACT planes use saturated Sigmoid(64*(idx-8a+0.5)) which is exactly
  0/1 in bf16, so no sign-correction anywhere
- Gram: C[(b,g),(a,g')] += SL[:,j].T @ SH[:,j] over 128 f-interleaved
  chunks; mask out g!=g', halving-add ladder over g' (idle Pool), then
  e8.T @ cred -> C2[b, a] per sample written into a shared PSUM tile
- 2nd difference of C2 + MLP tail, all b-major

Scheduling (the critical part, engine queues run in emission order):
- per iteration: dma(s+2); planes(s); idx(s+1); Gram(s); epilogue(s-1)
  so no engine queue head ever waits on a slower producer
- PE work/sample (6.87us) slightly exceeds every plane engine, keeping
  the PE continuously busy at full p-state (it is the pipeline pacer)
- all weights ride ONE blob DMA, dispatched after x(0)/x(1)
"""

import numpy as np

import concourse.bacc as bacc
import concourse.mybir as mybir
from concourse.mybir import AluOpType
from concourse.tile import TileContext
from concourse.bass_utils import run_bass_kernel_spmd

HIST_BINS = 64
VMAX = 255.0
SCALE = float(np.float32(HIST_BINS / VMAX))
B, H, W = 128, 512, 512
N_CORES = 8
SPC = B // N_CORES            # 16 samples per core
NPEL = H * W                  # 262144
P = 128
PF = NPEL // P                # 2048 free elems per partition
G = 8                         # f-columns per Gram matmul
NMM = PF // G                 # 128 matmuls per sample

F32 = mybir.dt.float32
I16 = mybir.dt.int16
BF16 = mybir.dt.bfloat16

POOL_HI = (2, 3)              # hi planes always built on GpSimd
ACT_HI = (5, 6, 7)            # hi planes always built on ScalarE
ALT_HI = 4                    # hi plane split ACT/VE by j-range
H1_VE = 236                   # VE builds j 0:H1_VE of hi plane 1, Pool rest
H4_ACT = 156                  # ACT builds j 0:H4_ACT of hi plane 4, VE rest
CUSHION_MM = 0                # extra PE matmuls/sample so the PE stays the
                              # (p-state-hot) pipeline pacer

# weight blob column layout: [mask 128 | e8 8 | abias 8 | dtb 8 | b2 1 | rest]
# rows 0..63 of 'rest' hold W1.T ([8a+b, j] 32 cols), rows 64..95 hold W2.T
# ([j, 128] needs 128 cols) - W2.T gets its own region instead.
BLOB_COLS = 64 + 8 + 8 + 8 + 1 + 256 + 128 + 1  # mask e8 abias dtb b2 w1 w2t b1


def build_kernel():
    nc = bacc.Bacc("TRN2", target_bir_lowering=False)

    x = nc.dram_tensor("x", [SPC, P, PF], F32, kind="ExternalInput")
    wblob = nc.dram_tensor("wblob", [P, BLOB_COLS], F32, kind="ExternalInput")
    y = nc.dram_tensor("y", [P, SPC], F32, kind="ExternalOutput")

    with TileContext(nc) as tc:
        with (
            tc.tile_pool(name="xp", bufs=3) as xp,
            tc.tile_pool(name="idxp", bufs=3) as idxp,
            tc.tile_pool(name="sm", bufs=1) as sm,
            tc.tile_pool(name="wk", bufs=3) as wk,
            tc.tile_pool(name="ps", bufs=3, space="PSUM") as ps,
            tc.tile_pool(name="psc", bufs=1, space="PSUM") as psc,
            tc.tile_pool(name="ps1", bufs=1, space="PSUM") as ps1,
        ):
            # a tiny dummy activation binds the ACT function-table load at
            # t~0 instead of on sample 0's critical path
            dummy_sb = sm.tile([1, 2], F32)
            nc.vector.memset(dummy_sb[:], 0.0)
            nc.scalar.activation(
                dummy_sb[:], dummy_sb[:],
                mybir.ActivationFunctionType.Sigmoid, bias=0.0, scale=1.0,
            )

            idxs = []
            xts = []

            def load_dma(s):
                xt = xp.tile([P, PF], F32, name=f"xt{s}", tag="xt")
                nc.sync.dma_start(out=xt[:], in_=x[s])
                xts.append(xt)

            # weight blob first (ACT planes gate the fill on abias), then
            # sample 0 in two halves, then x(1).
            # only the fill-critical weight columns (mask/e8/abias) load
            # ahead of sample 0; the big W1/W2 half rides later (tail-only)
            blob_sb = sm.tile([P, BLOB_COLS], F32)
            BSPLIT = 64 + 8 + 8
            nc.sync.dma_start(out=blob_sb[:, 0:BSPLIT], in_=wblob[:, 0:BSPLIT])
            xt0 = xp.tile([P, PF], F32, name="xt0", tag="xt")
            xts.append(xt0)
            HMM = NMM // 2
            nc.sync.dma_start(out=xt0[:, 0 : PF // 2], in_=x[0][:, 0 : PF // 2])
            nc.sync.dma_start(out=xt0[:, PF // 2 : PF], in_=x[0][:, PF // 2 : PF])
            load_dma(1)
            nc.sync.dma_start(
                out=blob_sb[:, BSPLIT:BLOB_COLS], in_=wblob[:, BSPLIT:BLOB_COLS]
            )
            c0 = 0
            mask_sb = blob_sb[0:64, c0 : c0 + 64]; c0 += 64
            e8_sb = blob_sb[0:64, c0 : c0 + 8]; c0 += 8
            abias_sb = blob_sb[:, c0 : c0 + 8]; c0 += 8
            dtb_sb = blob_sb[0:8, c0 : c0 + 8]; c0 += 8
            b2_sb = blob_sb[:, c0 : c0 + 1]; c0 += 1
            w1_sb = blob_sb[0:8, c0 : c0 + 256].rearrange(
                "b (a j) -> b a j", a=8
            ); c0 += 256   # [b, a, j]
            w2t_sb = blob_sb[0:32, c0 : c0 + 128]; c0 += 128
            b1_sb = blob_sb[0:32, c0 : c0 + 1]; c0 += 1

            # double-buffered step planes; ones plane written once each
            sh_tiles = [
                sm.tile([P, NMM, 8, G], BF16, name=f"sh{i}", tag=f"sh{i}")
                for i in range(2)
            ]
            sl_tiles = [
                sm.tile([P, NMM, 8, G], BF16, name=f"sl{i}", tag=f"sl{i}")
                for i in range(2)
            ]
            for i in range(2):
                nc.gpsimd.memset(sh_tiles[i][:, :, 0, :], 1.0)
                nc.vector.memset(sl_tiles[i][:, :, 0, :], 1.0)

            # C2 for all samples: [b, (s, a)], written by one matmul/sample
            t2b_ps = ps1.tile([8, SPC, 8], F32)

            los = []

            def emit_idx(s, e0=0, e1=PF):
                if len(idxs) <= s:
                    idxs.append(idxp.tile([P, PF], I16, name=f"idx{s}", tag="idx"))
                    los.append(idxp.tile([P, PF], I16, name=f"lo{s}", tag="lo"))
                nc.vector.tensor_scalar(
                    idxs[s][:, e0:e1], xts[s][:, e0:e1], SCALE, 0.5,
                    AluOpType.mult, AluOpType.subtract,
                )
                nc.vector.tensor_scalar(
                    los[s][:, e0:e1], idxs[s][:, e0:e1], 7, None,
                    AluOpType.bitwise_and,
                )

            def emit_planes(s, j0=0, j1=NMM, h4_act=None):
                if h4_act is None:
                    h4_act = H4_ACT
                idx_v = idxs[s][:].rearrange("p (j g) -> p j g", g=G)
                lo_v = los[s][:].rearrange("p (j g) -> p j g", g=G)
                SH = sh_tiles[s % 2]
                SL = sl_tiles[s % 2]
                # hi planes (idx >= 8a) -> SH[:, :, a, :]; planes 1 and 4
                # are split by j-range across engines for exact balance
                s1 = min(max(H1_VE, j0), j1)
                if s1 > j0:
                    nc.vector.tensor_scalar(
                        SH[:, j0:s1, 1, :], idx_v[:, j0:s1, :], 8.0, None,
                        AluOpType.is_ge,
                    )
                if j1 > s1:
                    nc.gpsimd.tensor_scalar(
                        SH[:, s1:j1, 1, :], idx_v[:, s1:j1, :], 8.0, None,
                        AluOpType.is_ge,
                    )
                for a in POOL_HI:
                    nc.gpsimd.tensor_scalar(
                        SH[:, j0:j1, a, :], idx_v[:, j0:j1, :], float(8 * a),
                        None, AluOpType.is_ge,
                    )
                for a in ACT_HI:
                    # saturated sigmoid: exactly 0/1 in bf16 for integer idx
                    nc.scalar.activation(
                        SH[:, j0:j1, a, :],
                        idx_v[:, j0:j1, :],
                        mybir.ActivationFunctionType.Sigmoid,
                        bias=abias_sb[:, a : a + 1],
                        scale=64.0,
                    )
                s4 = min(max(h4_act, j0), j1)
                if s4 > j0:
                    nc.scalar.activation(
                        SH[:, j0:s4, ALT_HI, :],
                        idx_v[:, j0:s4, :],
                        mybir.ActivationFunctionType.Sigmoid,
                        bias=abias_sb[:, ALT_HI : ALT_HI + 1],
                        scale=64.0,
                    )
                if j1 > s4:
                    nc.vector.tensor_scalar(
                        SH[:, s4:j1, ALT_HI, :], idx_v[:, s4:j1, :],
                        float(8 * ALT_HI), None, AluOpType.is_ge,
                    )
                # lo planes (lo >= b), one single-op pass each
                for b in range(1, 8):
                    nc.vector.tensor_scalar(
                        SL[:, j0:j1, b, :], lo_v[:, j0:j1, :], float(b), None,
                        AluOpType.is_ge,
                    )

            cps_tiles = {}

            def emit_gram(s):
                SH = sh_tiles[s % 2]
                SL = sl_tiles[s % 2]
                c_ps = ps.tile([64, 64], F32, tag="cps")
                cps_tiles[s] = c_ps
                for j in range(NMM):
                    nc.tensor.matmul(
                        c_ps[:],
                        SL[:, j].rearrange("p b g -> p (b g)"),
                        SH[:, j].rearrange("p a g -> p (a g)"),
                        start=(j == 0),
                        stop=(j == NMM - 1),
                    )
                if CUSHION_MM:
                    cu_ps = psc.tile([64, 64], F32, tag="cush")
                    for j in range(CUSHION_MM):
                        nc.tensor.matmul(
                            cu_ps[:],
                            SL[:, j].rearrange("p b g -> p (b g)"),
                            SH[:, j].rearrange("p a g -> p (a g)"),
                            start=(j == 0),
                            stop=(j == CUSHION_MM - 1),
                        )

            def emit_epilogue(s, ladder_ve=False):
                c_ps = cps_tiles[s]
                # mask g != g' cross terms (VE: GpSimd cannot read PSUM)
                cm = wk.tile([64, 8, G], F32, tag="cm")
                nc.vector.tensor_tensor(
                    cm[:].rearrange("p a g -> p (a g)"), c_ps[:], mask_sb[:],
                    AluOpType.mult,
                )
                if ladder_ve:
                    # last sample: the g'-reduce rides the (idle) PE as 8
                    # accumulating [8x8] matmuls instead of the add ladder,
                    # cutting the ladder + its sem hops from the tail
                    for k in range(G):
                        nc.tensor.matmul(
                            t2b_ps[:, s, :], e8_sb[:], cm[:, :, k],
                            start=(k == 0), stop=(k == G - 1),
                        )
                    return
                # halving-add ladder over g' on the idle GpSimd engine
                ch4 = wk.tile([64, 8, 4], F32, tag="ch4")
                nc.gpsimd.tensor_tensor(
                    ch4[:], cm[:, :, 0:4], cm[:, :, 4:8], AluOpType.add
                )
                ch2 = wk.tile([64, 8, 2], F32, tag="ch2")
                nc.gpsimd.tensor_tensor(
                    ch2[:], ch4[:, :, 0:2], ch4[:, :, 2:4], AluOpType.add
                )
                cred = wk.tile([64, 8], F32, tag="cred")
                nc.gpsimd.tensor_tensor(
                    cred[:], ch2[:, :, 0], ch2[:, :, 1], AluOpType.add
                )
                # C2[b, a] for this sample straight into the shared PSUM tile
                nc.tensor.matmul(
                    t2b_ps[:, s, :], e8_sb[:], cred[:], start=True, stop=True
                )

            # emission order is engine-queue order: keep VE planes ahead of
            # the next idx, and the PE epilogue of s-1 behind Gram(s), so no
            # engine queue head ever waits on a slower producer.
            # sample 0 in two halves behind two half-DMAs (shorter fill);
            # its share of plane 4 goes to VE (the fill is ACT-gated)
            emit_idx(0, 0, PF // 2)
            emit_planes(0, 0, HMM)
            emit_idx(0, PF // 2, PF)
            emit_planes(0, HMM, NMM)
            for s in range(SPC):
                if s + 2 < SPC:
                    load_dma(s + 2)
                if s > 0:
                    emit_planes(s)
                if s + 1 < SPC:
                    emit_idx(s + 1)
                emit_gram(s)
                if s > 0:
                    emit_epilogue(s - 1)

            # ---- tail: the whole MLP runs twice, samples 0..14 overlap
            # the last Gram/epilogue; only sample 15's column rides the
            # serial late chain ----
            t2b_sb = sm.tile([8, SPC, 8], F32)
            h1_ps = ps1.tile([32, SPC], F32)
            h1r_sb = sm.tile([32, SPC], F32)
            out_ps = ps1.tile([P, SPC], F32)
            out_sb = sm.tile([P, SPC], F32)
            SL15 = SPC - 1

            def emit_mlp_tail(s0, s1):
                nc.scalar.activation(
                    t2b_sb[:, s0:s1].rearrange("p s a -> p (s a)"),
                    t2b_ps[:, s0:s1].rearrange("p s a -> p (s a)"),
                    mybir.ActivationFunctionType.Copy,
                    bias=0.0,
                    scale=1.0,
                )
                # h1 = sum_a What[:, a, :].T @ C2[:, (s, a)]; both second-
                # difference matrices and the 1/N are folded into What
                for a in range(8):
                    nc.tensor.matmul(
                        h1_ps[:, s0:s1],
                        w1_sb[:, a, :],
                        t2b_sb[:, s0:s1, a],
                        start=(a == 0),
                        stop=(a == 7),
                    )
                nc.scalar.activation(
                    h1r_sb[:, s0:s1], h1_ps[:, s0:s1],
                    mybir.ActivationFunctionType.Relu, bias=b1_sb, scale=1.0,
                )
                nc.tensor.matmul(
                    out_ps[:, s0:s1], w2t_sb, h1r_sb[:, s0:s1],
                    start=True, stop=True,
                )
                nc.scalar.activation(
                    out_sb[:, s0:s1], out_ps[:, s0:s1],
                    mybir.ActivationFunctionType.Identity, bias=b2_sb, scale=1.0,
                )
                nc.sync.dma_start(out=y[:, s0:s1], in_=out_sb[:, s0:s1])

            emit_mlp_tail(0, SL15)
            emit_epilogue(SL15, ladder_ve=True)
            emit_mlp_tail(SL15, SPC)

    nc.compile()
    return nc


_NC_CACHE = {}


def kernel(grad_map, W1, b1, W2, b2, _trace=False):
    grad_map = np.ascontiguousarray(grad_map, dtype=np.float32)
    W1 = np.asarray(W1, dtype=np.float32)
    b1 = np.asarray(b1, dtype=np.float32)
    W2 = np.asarray(W2, dtype=np.float32)
    b2 = np.asarray(b2, dtype=np.float32)

    if "nc" not in _NC_CACHE:
        _NC_CACHE["nc"] = build_kernel()
    nc = _NC_CACHE["nc"]

    blob = np.zeros((P, BLOB_COLS), np.float32)
    c0 = 0
    # mask[(b,g), (a,g')] = delta_{g,g'}
    blob[0:64, c0 : c0 + 64] = np.kron(
        np.ones((8, 8), np.float32), np.eye(G, dtype=np.float32)
    ); c0 += 64
    # e8[(b,g), b'] = delta_{b,b'}
    blob[0:64, c0 : c0 + 8] = np.kron(
        np.eye(8, dtype=np.float32), np.ones((G, 1), np.float32)
    ); c0 += 8
    # sigmoid bias per hi plane a: 64*(0.5 - 8a)
    blob[:, c0 : c0 + 8] = np.array(
        [64.0 * (0.5 - 8.0 * a) for a in range(8)], np.float32
    )[None, :]; c0 += 8
    # dtb slot kept for layout compatibility (no longer used on-device)
    dbm = np.eye(8, dtype=np.float32) - np.eye(8, k=1, dtype=np.float32)
    blob[0:8, c0 : c0 + 8] = (dbm / np.float32(NPEL)).T; c0 += 8
    blob[:, c0] = b2; c0 += 1
    # What[b, a, j] = sum_{b',a'} DB[b',b] DA[a',a] W1[j, 8a'+b'] / N:
    # h1 = sum_{b,a} What[b,a,j] C2[a,b] equals W1 @ histn (both second
    # differences of C2 folded into the weights)
    w1r = W1.T.reshape(8, 8, 32).transpose(1, 0, 2)      # [b', a', j]
    what = np.einsum("ki,lj,klm->ijm", dbm, dbm, w1r) / np.float32(NPEL)
    blob[0:8, c0 : c0 + 256] = what.reshape(8, 256).astype(np.float32); c0 += 256
    blob[0:32, c0 : c0 + 128] = W2.T; c0 += 128
    blob[0:32, c0] = b1; c0 += 1
    assert c0 == BLOB_COLS

    xs = grad_map.reshape(N_CORES, SPC, P, PF)
    in_maps = [
        {"x": np.ascontiguousarray(xs[c]), "wblob": blob} for c in range(N_CORES)
    ]

    res = run_bass_kernel_spmd(
        nc, in_maps, core_ids=list(range(N_CORES)), trace=_trace
    )
    out = np.concatenate([r["y"].T for r in res.results], axis=0)
    if _trace:
        return out, res
    return out
